# revision 12
# baseline (speedup 1.0000x reference)
"""CLUB loss kernel for Trainium2, 8 NeuronCores (SPMD data-parallel).

Math: with flat_x (N,d), iv = exp(-p_logvar):
  positive_i = -0.5 * sum_d (x_i - mu_i)^2 * iv_i
  negative_i = -0.5 * sum_d iv_i * (ex2 - 2 mu_i ex + mu_i^2)
  loss = mean_i(positive_i - negative_i)
Decomposed into global sums (single pass over data):
  sx[d], sxx[d], A[d]=sum iv, B2[d]=sum iv*mu, Ta=sum iv*x^2, Tb=sum iv*mu*x
  loss = -0.5/N * [(Ta - 2 Tb) - dot(sxx,A)/N + dot(sx,2*B2)/N]

v2 design (vs v1 which was DVE/GPSIMD-bound and DMA-window-bound):
 - All DRAM loads are fully contiguous (8KB per partition): mu/lv land
   "i-major permuted" (partition p holds rows 16p..16p+15 of its 2048-row
   tile). All reductions are permutation-invariant over rows, so any
   row->partition assignment works as long as the x side matches.
 - x (d-major natural) is cast to fp16 and PE-transposed per 128-col group
   with column stride 16, which reproduces exactly the same row permutation
   (partition k of transpose block r = row 16k+r of the tile).
 - The coupled sums run on the PE as per-block [128x128] matmuls
   accumulated in PSUM over all 64 blocks:
     M1: lhsT=iv_blk,  rhs=[xsqT_blk | ones] -> diag = Ta partials, col128 = A
     M2: lhsT=j_blk,   rhs=[xT_blk   | ones] -> diag = Tb partials, col128 = B2
   fp16 operands (1 cyc/col); fp32 PSUM accumulation. fp16 keeps the
   cancellation-amplified error at ~1e-3 (measured vs reference), bf16 would
   be ~2.4e-3 and fp32 matmul is 4x slower.
 - sxx via GPSIMD partition-reduce (axis=C) of xsqT; sx via DVE free-dim
   reduce of the natural d-major x (engine balancing).
Device emits a per-core (128,6) stats block; host does the O(d) combine.
"""

import numpy as np

B, D, H, W = 16, 128, 64, 64
N = B * H * W            # 65536
NCORES = 8
BPC = B // NCORES        # 2 batches per core
HW = H * W               # 4096
ROWS = BPC * HW          # 8192 rows per core
TILE = 2048              # rows per tile
NT = ROWS // TILE        # 4 tiles per core
BLK = TILE // 128        # 16 transpose blocks per tile
NBLK = ROWS // 128       # 64 blocks per core

_CACHE = {}


def _build_nc(stats_output=True):
    import concourse.bass as bass
    import concourse.bacc as bacc
    import concourse.mybir as mybir
    from concourse import masks
    from concourse.tile import TileContext

    f32 = mybir.dt.float32
    f16 = mybir.dt.float16
    ALU = mybir.AluOpType
    AF = mybir.ActivationFunctionType
    AX = mybir.AxisListType

    nc = bacc.Bacc(num_devices=NCORES)
    x_in = nc.dram_tensor("x", [BPC, D, HW], f32, kind="ExternalInput")
    mu_in = nc.dram_tensor("p_mu", [ROWS, D], f32, kind="ExternalInput")
    lv_in = nc.dram_tensor("p_logvar", [ROWS, D], f32, kind="ExternalInput")
    stats_out = nc.dram_tensor("stats", [128, 6], f32, kind="ExternalOutput")

    with TileContext(nc) as tc:
        with (
            tc.tile_pool(name="const", bufs=1) as constp,
            tc.tile_pool(name="slabs", bufs=4) as slabs,
            tc.tile_pool(name="big", bufs=1) as big,
            tc.tile_pool(name="work", bufs=2) as work,
            tc.tile_pool(name="stats", bufs=1) as stats,
            tc.tile_pool(name="ps", bufs=2, space="PSUM") as psp,
            tc.tile_pool(name="psacc", bufs=1, space="PSUM") as psacc,
        ):
            ident16 = constp.tile([128, 128], f16, name="ident16")
            masks.make_identity(nc, ident16[:])
            identf = constp.tile([128, 128], f32, name="identf")
            masks.make_identity(nc, identf[:])
            onecell = constp.tile([1, 1], f32, name="onecell")
            nc.vector.memset(onecell[:], 1.0)
            ones_col = constp.tile([128, 1], f16, name="ones_col")
            nc.vector.memset(ones_col[:], 1.0)

            # persistent transposed-x layouts with a ones column every 129
            xTs = big.tile([128, NBLK * 129], f16, name="xTs")
            xsqTs = big.tile([128, NBLK * 129], f16, name="xsqTs")
            xTs_v = xTs[:].rearrange("p (n c) -> p n c", c=129)
            xsqTs_v = xsqTs[:].rearrange("p (n c) -> p n c", c=129)
            nc.vector.memset(xTs_v[:, :, 128:129], 1.0)
            nc.vector.memset(xsqTs_v[:, :, 128:129], 1.0)

            # PSUM accumulators for the coupled matmul streams
            P1 = psacc.tile([128, 129], f32, name="P1")
            P2 = psacc.tile([128, 129], f32, name="P2")
            P3 = psacc.tile([1, 129], f32, name="P3")

            sx_cols = stats.tile([128, 2 * NT], f32, name="sx_cols")

            # per-tile state kept across the 1-half-tile software pipeline lag
            ivs, jjs, psxs = {}, {}, {}

            def emit_mm(u):
                """M1/M2/M3 matmuls for half-tile u (blocks 8u..8u+8)."""
                t = u // 2
                iv, jj = ivs[t], jjs[t]
                for k in range(8):
                    blk = u * 8 + k          # global block in 0..63
                    r = (blk % BLK)          # block index within tile
                    first = blk == 0
                    last = blk == NBLK - 1
                    nc.tensor.matmul(
                        P1[:], iv[:, r * 128:(r + 1) * 128],
                        xsqTs_v[:, blk, :], start=first, stop=last,
                        skip_group_check=True)
                    nc.tensor.matmul(
                        P2[:], jj[:, r * 128:(r + 1) * 128],
                        xTs_v[:, blk, :], start=first, stop=last,
                        skip_group_check=True)
                    nc.tensor.matmul(
                        P3[:], ones_col[:],
                        xsqTs_v[:, blk, :], start=first, stop=last,
                        skip_group_check=True)

            for t in range(NT):
                b, h = t // 2, t % 2
                x_t = slabs.tile([128, TILE], f32, tag="x_t", name="x_t")
                mu_t = slabs.tile([128, TILE], f32, tag="mu_t", name="mu_t")
                lv_t = slabs.tile([128, TILE], f32, tag="lv_t", name="lv_t")
                r0 = t * TILE
                # lv first: the lv -> exp -> jj chain is the slowest
                # consumer path, x last: its consumers are fast per-half
                nc.sync.dma_start(
                    out=lv_t[:],
                    in_=lv_in[r0:r0 + TILE, :].rearrange(
                        "(p r) d -> p (r d)", p=128))
                nc.sync.dma_start(
                    out=mu_t[:],
                    in_=mu_in[r0:r0 + TILE, :].rearrange(
                        "(p r) d -> p (r d)", p=128))
                nc.sync.dma_start(out=x_t[:],
                                  in_=x_in[b, :, h * TILE:(h + 1) * TILE])

                iv = work.tile([128, TILE], f16, tag="iv", name="iv")
                jj = work.tile([128, TILE], f16, tag="jj", name="jj")
                xb = work.tile([128, TILE], f16, tag="xb", name="xb")
                ivs[t], jjs[t] = iv, jj
                HT = TILE // 2
                for hh in range(2):
                    sl = slice(hh * HT, (hh + 1) * HT)
                    # ACT: iv = exp(-lv)  (f32 -> f16)
                    nc.scalar.activation(iv[:, sl], lv_t[:, sl], AF.Exp,
                                         bias=0.0, scale=-1.0)
                    # GPSIMD: j = iv * mu (mixed f16*f32 -> f16; DVE's
                    # mixed path is a 131 cyc/elem microcode disaster)
                    nc.gpsimd.tensor_tensor(jj[:, sl], iv[:, sl],
                                            mu_t[:, sl], ALU.mult)
                    # DVE: sx partial (d-major, f32)
                    nc.vector.tensor_reduce(sx_cols[:, 2 * t + hh:
                                                    2 * t + hh + 1],
                                            x_t[:, sl], axis=AX.X, op=ALU.add)
                # ACT: xb = fp16(x)  (DVE CAST is ~4.4 cyc/elem; the
                # transposes read stride-16 columns so they need all of xb)
                nc.scalar.activation(xb[:], x_t[:], AF.Copy)

                # stride-16 column view of xb: xb_g[:, k, r] = xb[:, 16k+r]
                xb_g = xb[:].rearrange("p (k s) -> p k s", s=16)
                for hh in range(2):
                    u = t * 2 + hh
                    psx = psp.tile([128, 1024], f16, tag="psx", name="psx")
                    psxs[u] = psx
                    for k in range(8):
                        r = hh * 8 + k
                        nc.tensor.transpose(psx[:, k * 128:(k + 1) * 128],
                                            xb_g[:, :, r], ident16[:])
                    blk0 = u * 8
                    # ACT: squares into the ones-strided layout
                    nc.scalar.activation(
                        xsqTs_v[:, blk0:blk0 + 8, 0:128], psx[:], AF.Square)
                    # DVE: plain copy into the ones-strided layout
                    nc.vector.tensor_copy(
                        xTs_v[:, blk0:blk0 + 8, 0:128], psx[:])
                    if u >= 1:
                        emit_mm(u - 1)

            emit_mm(2 * NT - 1)

            # ---- wrap-up: fold everything into g[128, 6] ----
            g = stats.tile([128, 6], f32, name="g")
            scratch = stats.tile([128, 128], f32, name="scratch")
            # sx
            nc.vector.tensor_reduce(g[:, 0:1], sx_cols[:], axis=AX.X,
                                    op=ALU.add)
            # A, B2 from the ones columns
            nc.vector.tensor_copy(g[:, 1:2], P1[:, 128:129])
            nc.vector.tensor_copy(g[:, 2:3], P2[:, 128:129])
            # Ta, Tb from the diagonals
            nc.vector.tensor_tensor(scratch[:], P1[:, 0:128], identf[:],
                                    ALU.mult)
            nc.vector.tensor_reduce(g[:, 3:4], scratch[:], axis=AX.X,
                                    op=ALU.add)
            nc.vector.tensor_tensor(scratch[:], P2[:, 0:128], identf[:],
                                    ALU.mult)
            nc.vector.tensor_reduce(g[:, 4:5], scratch[:], axis=AX.X,
                                    op=ALU.add)
            # sxx: P3 row [1,128] -> SBUF, then to a column via the PE
            srow = stats.tile([1, 128], f32, name="srow")
            nc.vector.tensor_copy(srow[:], P3[0:1, 0:128])
            psC = psacc.tile([128, 1], f32, name="psC")
            nc.tensor.matmul(psC[:], srow[:], onecell[:], start=True,
                             stop=True)
            nc.vector.tensor_copy(g[:, 5:6], psC[:])

            nc.sync.dma_start(out=stats_out[:], in_=g[:])

    return nc


MODE = "host"


def get_nc(use_collective=True, stats_output=True):
    key = ("nc_v2",)
    if key not in _CACHE:
        nc = _build_nc()
        if not nc.is_finalized():
            nc.finalize()
        _CACHE[key] = nc
    return _CACHE[key]


def make_in_maps(x, p_mu, p_logvar):
    x = np.ascontiguousarray(np.asarray(x, dtype=np.float32))
    p_mu = np.ascontiguousarray(np.asarray(p_mu, dtype=np.float32))
    p_logvar = np.ascontiguousarray(np.asarray(p_logvar, dtype=np.float32))
    in_maps = []
    for c in range(NCORES):
        in_maps.append({
            "x": np.ascontiguousarray(
                x[c * BPC:(c + 1) * BPC].reshape(BPC, D, HW)),
            "p_mu": np.ascontiguousarray(p_mu[c * ROWS:(c + 1) * ROWS]),
            "p_logvar": np.ascontiguousarray(
                p_logvar[c * ROWS:(c + 1) * ROWS]),
        })
    return in_maps


def kernel(x, p_mu, p_logvar):
    from concourse.bass_utils import run_bass_kernel_spmd

    in_maps = make_in_maps(x, p_mu, p_logvar)
    nc = get_nc()
    res = run_bass_kernel_spmd(nc, in_maps, list(range(NCORES)))
    s = np.zeros((128, 6), dtype=np.float64)
    for c in range(NCORES):
        s += np.asarray(res.results[c]["stats"], dtype=np.float64)
    sx, A, B2p, Ta, Tb, sxx = (s[:, k] for k in range(6))
    T = Ta.sum() - 2.0 * Tb.sum()
    loss = -0.5 / N * (T - sxx.dot(A) / N + sx.dot(2.0 * B2p) / N)
    return np.asarray(loss, dtype=np.float32).reshape(())


# revision 15
# speedup vs baseline: 1.1112x; 1.1112x over previous
"""CLUB loss kernel for Trainium2, 8 NeuronCores (SPMD data-parallel).

Math: with flat_x (N,d), iv = exp(-p_logvar):
  positive_i = -0.5 * sum_d (x_i - mu_i)^2 * iv_i
  negative_i = -0.5 * sum_d iv_i * (ex2 - 2 mu_i ex + mu_i^2)
  loss = mean_i(positive_i - negative_i)
Decomposed into global sums (single pass over data):
  sx[d], sxx[d], A[d]=sum iv, B2[d]=sum iv*mu, Ta=sum iv*x^2, Tb=sum iv*mu*x
  loss = -0.5/N * [(Ta - 2 Tb) - dot(sxx,A)/N + dot(sx,2*B2)/N]

v2 design (vs v1 which was DVE/GPSIMD-bound and DMA-window-bound):
 - All DRAM loads are fully contiguous (8KB per partition): mu/lv land
   "i-major permuted" (partition p holds rows 16p..16p+15 of its 2048-row
   tile). All reductions are permutation-invariant over rows, so any
   row->partition assignment works as long as the x side matches.
 - x (d-major natural) is cast to fp16 and PE-transposed per 128-col group
   with column stride 16, which reproduces exactly the same row permutation
   (partition k of transpose block r = row 16k+r of the tile).
 - The coupled sums run on the PE as per-block [128x128] matmuls
   accumulated in PSUM over all 64 blocks:
     M1: lhsT=iv_blk,  rhs=[xsqT_blk | ones] -> diag = Ta partials, col128 = A
     M2: lhsT=j_blk,   rhs=[xT_blk   | ones] -> diag = Tb partials, col128 = B2
   fp16 operands (1 cyc/col); fp32 PSUM accumulation. fp16 keeps the
   cancellation-amplified error at ~1e-3 (measured vs reference), bf16 would
   be ~2.4e-3 and fp32 matmul is 4x slower.
 - sxx via GPSIMD partition-reduce (axis=C) of xsqT; sx via DVE free-dim
   reduce of the natural d-major x (engine balancing).
Device emits a per-core (128,6) stats block; host does the O(d) combine.
"""

import numpy as np

B, D, H, W = 16, 128, 64, 64
N = B * H * W            # 65536
NCORES = 8
BPC = B // NCORES        # 2 batches per core
HW = H * W               # 4096
ROWS = BPC * HW          # 8192 rows per core
TILE = 2048              # rows per tile
NT = ROWS // TILE        # 4 tiles per core
BLK = TILE // 128        # 16 transpose blocks per tile
NBLK = ROWS // 128       # 64 blocks per core

_CACHE = {}


def _build_nc(stats_output=True):
    import concourse.bass as bass
    import concourse.bacc as bacc
    import concourse.mybir as mybir
    from concourse import masks
    from concourse.tile import TileContext

    f32 = mybir.dt.float32
    f16 = mybir.dt.float16
    ALU = mybir.AluOpType
    AF = mybir.ActivationFunctionType
    AX = mybir.AxisListType

    nc = bacc.Bacc(num_devices=NCORES)
    x_in = nc.dram_tensor("x", [BPC, D, HW], f32, kind="ExternalInput")
    mu_in = nc.dram_tensor("p_mu", [ROWS, D], f32, kind="ExternalInput")
    lv_in = nc.dram_tensor("p_logvar", [ROWS, D], f32, kind="ExternalInput")
    stats_out = nc.dram_tensor("stats", [128, 6], f32, kind="ExternalOutput")

    with TileContext(nc) as tc:
        with (
            tc.tile_pool(name="const", bufs=1) as constp,
            tc.tile_pool(name="slabs", bufs=4) as slabs,
            tc.tile_pool(name="big", bufs=1) as big,
            tc.tile_pool(name="work", bufs=2) as work,
            tc.tile_pool(name="stats", bufs=1) as stats,
            tc.tile_pool(name="ps", bufs=2, space="PSUM") as psp,
            tc.tile_pool(name="psacc", bufs=1, space="PSUM") as psacc,
        ):
            # issue every input DMA first: the triggers have no deps, and
            # the 16 DMA engines stream ~12.6MB for ~32us — the earlier
            # they start, the earlier the whole pipeline finishes
            slabs_xml = []
            for t in range(NT):
                b, h = t // 2, t % 2
                x_t = slabs.tile([128, TILE], f32, tag="x_t", name="x_t")
                mu_t = slabs.tile([128, TILE], f32, tag="mu_t", name="mu_t")
                lv_t = slabs.tile([128, TILE], f32, tag="lv_t", name="lv_t")
                r0 = t * TILE
                nc.sync.dma_start(out=x_t[:],
                                  in_=x_in[b, :, h * TILE:(h + 1) * TILE])
                nc.sync.dma_start(
                    out=lv_t[:],
                    in_=lv_in[r0:r0 + TILE, :].rearrange(
                        "(p r) d -> p (r d)", p=128))
                nc.sync.dma_start(
                    out=mu_t[:],
                    in_=mu_in[r0:r0 + TILE, :].rearrange(
                        "(p r) d -> p (r d)", p=128))
                slabs_xml.append((x_t, mu_t, lv_t))

            ident16 = constp.tile([128, 128], f16, name="ident16")
            masks.make_identity(nc, ident16[:])
            identf = constp.tile([128, 128], f32, name="identf")
            masks.make_identity(nc, identf[:])
            onecell = constp.tile([1, 1], f32, name="onecell")
            nc.vector.memset(onecell[:], 1.0)
            ones_col = constp.tile([128, 1], f16, name="ones_col")
            nc.vector.memset(ones_col[:], 1.0)

            # persistent transposed-x layouts with a ones column every 129
            xTs = big.tile([128, NBLK * 129], f16, name="xTs")
            xsqTs = big.tile([128, NBLK * 129], f16, name="xsqTs")
            xTs_v = xTs[:].rearrange("p (n c) -> p n c", c=129)
            xsqTs_v = xsqTs[:].rearrange("p (n c) -> p n c", c=129)
            nc.vector.memset(xTs_v[:, :, 128:129], 1.0)
            nc.vector.memset(xsqTs_v[:, :, 128:129], 1.0)

            # PSUM accumulators for the coupled matmul streams
            P1 = psacc.tile([128, 129], f32, name="P1")
            P2 = psacc.tile([128, 129], f32, name="P2")
            P3 = psacc.tile([1, 129], f32, name="P3")

            sx_cols = stats.tile([128, 2 * NT], f32, name="sx_cols")

            # per-tile state kept across the 1-half-tile software pipeline lag
            ivs, jjs, psxs = {}, {}, {}

            def emit_mm(u):
                """M1/M2/M3 matmuls for half-tile u (blocks 8u..8u+8)."""
                t = u // 2
                iv, jj = ivs[t], jjs[t]
                for k in range(8):
                    blk = u * 8 + k          # global block in 0..63
                    r = (blk % BLK)          # block index within tile
                    first = blk == 0
                    last = blk == NBLK - 1
                    nc.tensor.matmul(
                        P1[:], iv[:, r * 128:(r + 1) * 128],
                        xsqTs_v[:, blk, :], start=first, stop=last,
                        skip_group_check=True)
                    nc.tensor.matmul(
                        P2[:], jj[:, r * 128:(r + 1) * 128],
                        xTs_v[:, blk, :], start=first, stop=last,
                        skip_group_check=True)
                    nc.tensor.matmul(
                        P3[:], ones_col[:],
                        xsqTs_v[:, blk, :], start=first, stop=last,
                        skip_group_check=True)

            for t in range(NT):
                x_t, mu_t, lv_t = slabs_xml[t]
                iv = work.tile([128, TILE], f16, tag="iv", name="iv")
                jj = work.tile([128, TILE], f16, tag="jj", name="jj")
                xb = work.tile([128, TILE], f16, tag="xb", name="xb")
                ivs[t], jjs[t] = iv, jj
                HT = TILE // 2
                for hh in range(2):
                    sl = slice(hh * HT, (hh + 1) * HT)
                    # ACT: iv = exp(-lv)  (f32 -> f16)
                    nc.scalar.activation(iv[:, sl], lv_t[:, sl], AF.Exp,
                                         bias=0.0, scale=-1.0)
                    # GPSIMD: j = iv * mu (mixed f16*f32 -> f16; DVE's
                    # mixed path is a 131 cyc/elem microcode disaster)
                    nc.gpsimd.tensor_tensor(jj[:, sl], iv[:, sl],
                                            mu_t[:, sl], ALU.mult)
                    # DVE: sx partial (d-major, f32)
                    nc.vector.tensor_reduce(sx_cols[:, 2 * t + hh:
                                                    2 * t + hh + 1],
                                            x_t[:, sl], axis=AX.X, op=ALU.add)
                # ACT: xb = fp16(x)  (DVE CAST is ~4.4 cyc/elem; the
                # transposes read stride-16 columns so they need all of xb)
                nc.scalar.activation(xb[:], x_t[:], AF.Copy)

                # stride-16 column view of xb: xb_g[:, k, r] = xb[:, 16k+r]
                xb_g = xb[:].rearrange("p (k s) -> p k s", s=16)
                for hh in range(2):
                    u = t * 2 + hh
                    psx = psp.tile([128, 1024], f16, tag="psx", name="psx",
                                   bufs=3)
                    psxs[u] = psx
                    for k in range(8):
                        r = hh * 8 + k
                        nc.tensor.transpose(psx[:, k * 128:(k + 1) * 128],
                                            xb_g[:, :, r], ident16[:])
                    blk0 = u * 8
                    # ACT: squares into the ones-strided layout
                    nc.scalar.activation(
                        xsqTs_v[:, blk0:blk0 + 8, 0:128], psx[:], AF.Square)
                    # DVE: plain copy into the ones-strided layout
                    nc.vector.tensor_copy(
                        xTs_v[:, blk0:blk0 + 8, 0:128], psx[:])
                    if u >= 2:
                        emit_mm(u - 2)

            emit_mm(2 * NT - 2)
            emit_mm(2 * NT - 1)

            # ---- wrap-up: fold everything into g[128, 6] ----
            g = stats.tile([128, 6], f32, name="g")
            scratch = stats.tile([128, 128], f32, name="scratch")
            # sx
            nc.vector.tensor_reduce(g[:, 0:1], sx_cols[:], axis=AX.X,
                                    op=ALU.add)
            # A, B2 from the ones columns
            nc.vector.tensor_copy(g[:, 1:2], P1[:, 128:129])
            nc.vector.tensor_copy(g[:, 2:3], P2[:, 128:129])
            # Ta, Tb from the diagonals
            nc.vector.tensor_tensor(scratch[:], P1[:, 0:128], identf[:],
                                    ALU.mult)
            nc.vector.tensor_reduce(g[:, 3:4], scratch[:], axis=AX.X,
                                    op=ALU.add)
            nc.vector.tensor_tensor(scratch[:], P2[:, 0:128], identf[:],
                                    ALU.mult)
            nc.vector.tensor_reduce(g[:, 4:5], scratch[:], axis=AX.X,
                                    op=ALU.add)
            # sxx: P3 row [1,128] -> SBUF, then to a column via the PE
            srow = stats.tile([1, 128], f32, name="srow")
            nc.vector.tensor_copy(srow[:], P3[0:1, 0:128])
            psC = psacc.tile([128, 1], f32, name="psC")
            nc.tensor.matmul(psC[:], srow[:], onecell[:], start=True,
                             stop=True)
            nc.vector.tensor_copy(g[:, 5:6], psC[:])

            nc.sync.dma_start(out=stats_out[:], in_=g[:])

    return nc


MODE = "host"


def get_nc(use_collective=True, stats_output=True):
    key = ("nc_v2",)
    if key not in _CACHE:
        nc = _build_nc()
        if not nc.is_finalized():
            nc.finalize()
        _CACHE[key] = nc
    return _CACHE[key]


def make_in_maps(x, p_mu, p_logvar):
    x = np.ascontiguousarray(np.asarray(x, dtype=np.float32))
    p_mu = np.ascontiguousarray(np.asarray(p_mu, dtype=np.float32))
    p_logvar = np.ascontiguousarray(np.asarray(p_logvar, dtype=np.float32))
    in_maps = []
    for c in range(NCORES):
        in_maps.append({
            "x": np.ascontiguousarray(
                x[c * BPC:(c + 1) * BPC].reshape(BPC, D, HW)),
            "p_mu": np.ascontiguousarray(p_mu[c * ROWS:(c + 1) * ROWS]),
            "p_logvar": np.ascontiguousarray(
                p_logvar[c * ROWS:(c + 1) * ROWS]),
        })
    return in_maps


def kernel(x, p_mu, p_logvar):
    from concourse.bass_utils import run_bass_kernel_spmd

    in_maps = make_in_maps(x, p_mu, p_logvar)
    nc = get_nc()
    res = run_bass_kernel_spmd(nc, in_maps, list(range(NCORES)))
    s = np.zeros((128, 6), dtype=np.float64)
    for c in range(NCORES):
        s += np.asarray(res.results[c]["stats"], dtype=np.float64)
    sx, A, B2p, Ta, Tb, sxx = (s[:, k] for k in range(6))
    T = Ta.sum() - 2.0 * Tb.sum()
    loss = -0.5 / N * (T - sxx.dot(A) / N + sx.dot(2.0 * B2p) / N)
    return np.asarray(loss, dtype=np.float32).reshape(())


# revision 19
# speedup vs baseline: 1.2104x; 1.0893x over previous
"""CLUB loss kernel for Trainium2, 8 NeuronCores (SPMD data-parallel).

Math: with flat_x (N,d), iv = exp(-p_logvar):
  positive_i = -0.5 * sum_d (x_i - mu_i)^2 * iv_i
  negative_i = -0.5 * sum_d iv_i * (ex2 - 2 mu_i ex + mu_i^2)
  loss = mean_i(positive_i - negative_i)
Decomposed into global sums (single pass over data):
  sx[d], sxx[d], A[d]=sum iv, B2[d]=sum iv*mu, Ta=sum iv*x^2, Tb=sum iv*mu*x
  loss = -0.5/N * [(Ta - 2 Tb) - dot(sxx,A)/N + dot(sx,2*B2)/N]

v2 design (vs v1 which was DVE/GPSIMD-bound and DMA-window-bound):
 - All DRAM loads are fully contiguous (8KB per partition): mu/lv land
   "i-major permuted" (partition p holds rows 16p..16p+15 of its 2048-row
   tile). All reductions are permutation-invariant over rows, so any
   row->partition assignment works as long as the x side matches.
 - x (d-major natural) is cast to fp16 and PE-transposed per 128-col group
   with column stride 16, which reproduces exactly the same row permutation
   (partition k of transpose block r = row 16k+r of the tile).
 - The coupled sums run on the PE as per-block [128x128] matmuls
   accumulated in PSUM over all 64 blocks:
     M1: lhsT=iv_blk,  rhs=[xsqT_blk | ones] -> diag = Ta partials, col128 = A
     M2: lhsT=j_blk,   rhs=[xT_blk   | ones] -> diag = Tb partials, col128 = B2
   fp16 operands (1 cyc/col); fp32 PSUM accumulation. fp16 keeps the
   cancellation-amplified error at ~1e-3 (measured vs reference), bf16 would
   be ~2.4e-3 and fp32 matmul is 4x slower.
 - sxx via GPSIMD partition-reduce (axis=C) of xsqT; sx via DVE free-dim
   reduce of the natural d-major x (engine balancing).
Device emits a per-core (128,6) stats block; host does the O(d) combine.
"""

import numpy as np

B, D, H, W = 16, 128, 64, 64
N = B * H * W            # 65536
NCORES = 8
BPC = B // NCORES        # 2 batches per core
HW = H * W               # 4096
ROWS = BPC * HW          # 8192 rows per core
TILE = 2048              # rows per tile
NT = ROWS // TILE        # 4 tiles per core
BLK = TILE // 128        # 16 transpose blocks per tile
NBLK = ROWS // 128       # 64 blocks per core

_CACHE = {}


def _build_nc(stats_output=True):
    import concourse.bass as bass
    import concourse.bacc as bacc
    import concourse.mybir as mybir
    from concourse import masks
    from concourse.tile import TileContext

    f32 = mybir.dt.float32
    f16 = mybir.dt.float16
    ALU = mybir.AluOpType
    AF = mybir.ActivationFunctionType
    AX = mybir.AxisListType

    nc = bacc.Bacc(num_devices=NCORES)
    x_in = nc.dram_tensor("x", [BPC, D, HW], f32, kind="ExternalInput")
    mu_in = nc.dram_tensor("p_mu", [ROWS, D], f32, kind="ExternalInput")
    lv_in = nc.dram_tensor("p_logvar", [ROWS, D], f32, kind="ExternalInput")
    stats_out = nc.dram_tensor("stats", [128, 6], f32, kind="ExternalOutput")

    with TileContext(nc) as tc:
        with (
            tc.tile_pool(name="const", bufs=1) as constp,
            tc.tile_pool(name="slabs", bufs=4) as slabs,
            tc.tile_pool(name="big", bufs=1) as big,
            tc.tile_pool(name="work", bufs=2) as work,
            tc.tile_pool(name="stats", bufs=1) as stats,
            tc.tile_pool(name="ps", bufs=2, space="PSUM") as psp,
            tc.tile_pool(name="psacc", bufs=1, space="PSUM") as psacc,
        ):
            # issue every input DMA first: the triggers have no deps, and
            # the 16 DMA engines stream ~12.6MB for ~32us — the earlier
            # they start, the earlier the whole pipeline finishes
            slabs_xml = []
            for t in range(NT):
                b, h = t // 2, t % 2
                x_t = slabs.tile([128, TILE], f32, tag="x_t", name="x_t")
                mu_t = slabs.tile([128, TILE], f32, tag="mu_t", name="mu_t")
                lv_t = slabs.tile([128, TILE], f32, tag="lv_t", name="lv_t")
                r0 = t * TILE
                nc.sync.dma_start(out=x_t[:],
                                  in_=x_in[b, :, h * TILE:(h + 1) * TILE])
                nc.sync.dma_start(
                    out=mu_t[:],
                    in_=mu_in[r0:r0 + TILE, :].rearrange(
                        "(p r) d -> p (r d)", p=128))
                nc.sync.dma_start(
                    out=lv_t[:],
                    in_=lv_in[r0:r0 + TILE, :].rearrange(
                        "(p r) d -> p (r d)", p=128))
                slabs_xml.append((x_t, mu_t, lv_t))

            ident16 = constp.tile([128, 128], f16, name="ident16")
            masks.make_identity(nc, ident16[:])
            identf = constp.tile([128, 128], f32, name="identf")
            masks.make_identity(nc, identf[:])
            onecell = constp.tile([1, 1], f32, name="onecell")
            nc.vector.memset(onecell[:], 1.0)
            ones_col = constp.tile([128, 1], f16, name="ones_col")
            nc.vector.memset(ones_col[:], 1.0)

            # persistent transposed-x layouts with a ones column every 129
            xTs = big.tile([128, NBLK * 129], f16, name="xTs")
            xsqTs = big.tile([128, NBLK * 129], f16, name="xsqTs")
            xTs_v = xTs[:].rearrange("p (n c) -> p n c", c=129)
            xsqTs_v = xsqTs[:].rearrange("p (n c) -> p n c", c=129)
            nc.vector.memset(xTs_v[:, :, 128:129], 1.0)
            nc.vector.memset(xsqTs_v[:, :, 128:129], 1.0)

            # PSUM accumulators for the coupled matmul streams
            P1 = psacc.tile([128, 129], f32, name="P1")
            P2 = psacc.tile([128, 129], f32, name="P2")
            P3 = psacc.tile([1, 129], f32, name="P3")

            sx_cols = stats.tile([128, 2 * NT], f32, name="sx_cols")

            # per-tile state kept across the 1-half-tile software pipeline lag
            ivs, jjs, psxs = {}, {}, {}

            def emit_mm(u):
                """M1/M2/M3 matmuls for half-tile u (blocks 8u..8u+8)."""
                t = u // 2
                iv, jj = ivs[t], jjs[t]
                for k in range(8):
                    blk = u * 8 + k          # global block in 0..63
                    r = (blk % BLK)          # block index within tile
                    first = blk == 0
                    last = blk == NBLK - 1
                    nc.tensor.matmul(
                        P1[:], iv[:, r * 128:(r + 1) * 128],
                        xsqTs_v[:, blk, :], start=first, stop=last,
                        skip_group_check=True)
                    nc.tensor.matmul(
                        P2[:], jj[:, r * 128:(r + 1) * 128],
                        xTs_v[:, blk, :], start=first, stop=last,
                        skip_group_check=True)
                    nc.tensor.matmul(
                        P3[:], ones_col[:],
                        xsqTs_v[:, blk, :], start=first, stop=last,
                        skip_group_check=True)

            for t in range(NT):
                x_t, mu_t, lv_t = slabs_xml[t]
                iv = work.tile([128, TILE], f16, tag="iv", name="iv")
                jj = work.tile([128, TILE], f16, tag="jj", name="jj")
                xb = work.tile([128, TILE], f16, tag="xb", name="xb")
                ivs[t], jjs[t] = iv, jj
                # ACT: xb = fp16(x)  (DVE CAST is ~4.4 cyc/elem; the
                # transposes read stride-16 columns so they need all of xb)
                nc.scalar.activation(xb[:], x_t[:], AF.Copy)
                QT = TILE // 4
                for q in range(4):
                    sl = slice(q * QT, (q + 1) * QT)
                    # ACT: iv = exp(-lv)  (f32 -> f16).  ACT carries ONLY
                    # exp+xb so the last tile's exp isn't queued behind
                    # x-side work; quarters let jj chase the exp.
                    nc.scalar.activation(iv[:, sl], lv_t[:, sl], AF.Exp,
                                         bias=0.0, scale=-1.0)
                    # GPSIMD: j = iv * mu (mixed f16*f32 -> f16; DVE's
                    # mixed path is a 131 cyc/elem microcode disaster)
                    nc.gpsimd.tensor_tensor(jj[:, sl], iv[:, sl],
                                            mu_t[:, sl], ALU.mult)
                HT = TILE // 2
                for hh in range(2):
                    sl = slice(hh * HT, (hh + 1) * HT)
                    # DVE: sx partial (d-major, f32)
                    nc.vector.tensor_reduce(sx_cols[:, 2 * t + hh:
                                                    2 * t + hh + 1],
                                            x_t[:, sl], axis=AX.X, op=ALU.add)

                # stride-16 column view of xb: xb_g[:, k, r] = xb[:, 16k+r]
                xb_g = xb[:].rearrange("p (k s) -> p k s", s=16)
                for hh in range(2):
                    u = t * 2 + hh
                    psx = psp.tile([128, 1024], f16, tag="psx", name="psx",
                                   bufs=3)
                    psxs[u] = psx
                    for k in range(8):
                        r = hh * 8 + k
                        nc.tensor.transpose(psx[:, k * 128:(k + 1) * 128],
                                            xb_g[:, :, r], ident16[:])
                    # DVE: plain copy into the ones-strided layout
                    blk0 = u * 8
                    nc.vector.tensor_copy(
                        xTs_v[:, blk0:blk0 + 8, 0:128], psx[:])
                    if u >= 2:
                        # ACT: squares, emitted one tile late so they are
                        # never queued ahead of a later tile's exp (ACT is
                        # in-order; a square stalled on the PE would block
                        # the exp chain and serialize the tail)
                        b0 = (u - 2) * 8
                        nc.scalar.activation(
                            xsqTs_v[:, b0:b0 + 8, 0:128], psxs[u - 2][:],
                            AF.Square)
                        emit_mm(u - 2)

            for u in (2 * NT - 2, 2 * NT - 1):
                b0 = u * 8
                nc.scalar.activation(
                    xsqTs_v[:, b0:b0 + 8, 0:128], psxs[u][:], AF.Square)
                emit_mm(u)

            # ---- wrap-up: fold everything into g[128, 6] ----
            g = stats.tile([128, 6], f32, name="g")
            scratch = stats.tile([128, 128], f32, name="scratch")
            # sx
            nc.vector.tensor_reduce(g[:, 0:1], sx_cols[:], axis=AX.X,
                                    op=ALU.add)
            # A, B2 from the ones columns
            nc.vector.tensor_copy(g[:, 1:2], P1[:, 128:129])
            nc.vector.tensor_copy(g[:, 2:3], P2[:, 128:129])
            # Ta, Tb from the diagonals
            nc.vector.tensor_tensor(scratch[:], P1[:, 0:128], identf[:],
                                    ALU.mult)
            nc.vector.tensor_reduce(g[:, 3:4], scratch[:], axis=AX.X,
                                    op=ALU.add)
            nc.vector.tensor_tensor(scratch[:], P2[:, 0:128], identf[:],
                                    ALU.mult)
            nc.vector.tensor_reduce(g[:, 4:5], scratch[:], axis=AX.X,
                                    op=ALU.add)
            # sxx: P3 row [1,128] -> SBUF, then to a column via the PE
            srow = stats.tile([1, 128], f32, name="srow")
            nc.vector.tensor_copy(srow[:], P3[0:1, 0:128])
            psC = psacc.tile([128, 1], f32, name="psC")
            nc.tensor.matmul(psC[:], srow[:], onecell[:], start=True,
                             stop=True)
            nc.vector.tensor_copy(g[:, 5:6], psC[:])

            nc.sync.dma_start(out=stats_out[:], in_=g[:])

    return nc


MODE = "host"


def get_nc(use_collective=True, stats_output=True):
    key = ("nc_v2",)
    if key not in _CACHE:
        nc = _build_nc()
        if not nc.is_finalized():
            nc.finalize()
        _CACHE[key] = nc
    return _CACHE[key]


def make_in_maps(x, p_mu, p_logvar):
    x = np.ascontiguousarray(np.asarray(x, dtype=np.float32))
    p_mu = np.ascontiguousarray(np.asarray(p_mu, dtype=np.float32))
    p_logvar = np.ascontiguousarray(np.asarray(p_logvar, dtype=np.float32))
    in_maps = []
    for c in range(NCORES):
        in_maps.append({
            "x": np.ascontiguousarray(
                x[c * BPC:(c + 1) * BPC].reshape(BPC, D, HW)),
            "p_mu": np.ascontiguousarray(p_mu[c * ROWS:(c + 1) * ROWS]),
            "p_logvar": np.ascontiguousarray(
                p_logvar[c * ROWS:(c + 1) * ROWS]),
        })
    return in_maps


def kernel(x, p_mu, p_logvar):
    from concourse.bass_utils import run_bass_kernel_spmd

    in_maps = make_in_maps(x, p_mu, p_logvar)
    nc = get_nc()
    res = run_bass_kernel_spmd(nc, in_maps, list(range(NCORES)))
    s = np.zeros((128, 6), dtype=np.float64)
    for c in range(NCORES):
        s += np.asarray(res.results[c]["stats"], dtype=np.float64)
    sx, A, B2p, Ta, Tb, sxx = (s[:, k] for k in range(6))
    T = Ta.sum() - 2.0 * Tb.sum()
    loss = -0.5 / N * (T - sxx.dot(A) / N + sx.dot(2.0 * B2p) / N)
    return np.asarray(loss, dtype=np.float32).reshape(())
